# revision 5
# baseline (speedup 1.0000x reference)
"""GraphSAGE-style pooling aggregator kernel for Trainium2 (8 NeuronCores).

Computes, for full inputs:
    h      = relu(neighbor_features @ w_pool + bias_pool)   # (n*k, dim)
    pooled = max(h.reshape(n, k, dim), axis=1)              # (n, dim)
    out    = relu(concat([src, pooled], -1) @ w + bias)     # (n, out)

Sharding: data-parallel over nodes; weights replicated; no collectives.

v3 design notes (measured on HW):
  - bf16 phase-1 GEMM operands: the fp16 moving operand streams at 2
    PE-cycles/row even at full clock; bf16 streams at 1 (2x matmul rate).
    Host ships nbr^T as bf16 (10.24 MB/core vs 20.5 fp32).
  - PSUM drain (the bottleneck) is split across the only two engines with
    PSUM ports: DVE (0.96 GHz) + ACT (1.2 GHz), with a k-major host-side
    column permutation inside each 2-unit drain group so every tensor_max
    tree level is a flat 2D op (dense 16-bit 2x mode, no 3D-AP row
    restarts):
      pair scheme 'b': ACT copies unit B PSUM->SBUF fp16; DVE tensor_max
        (PSUM A, SBUF B) fuses A's drain with tree L1; then 4 flat levels.
      pair scheme 'a': ACT copies both units; L1 runs on DVE or GpSimd
        (3rd drain-adjacent engine, SBUF-only), then flat levels on DVE.
    The tail unit keeps k-minor layout and a direct DVE reduce_max.
  - Phase 2 (small) stays fp16 for precision: out^T = wtop^T src^T +
    wbot^T pooled^T, biases per-partition, host transposes the output.
  - ~7 dummy matmuls warm the PE HAM clock gate during the first DMA.
"""

import os

import numpy as np

N, K, DIM, OUT = 10000, 32, 128, 128
N_CORES = 8
NODES_PC = N // N_CORES          # 1250
ROWS_PC = NODES_PC * K           # 40000

UNIT = 2048                      # cols per PSUM tile (64 nodes)
MM = 512                         # cols per matmul (PSUM bank limit)
N_PAIRS = 9                      # 9 pairs x 4096 cols -> nodes 0..1151
SINGLE_NODE0 = 1152              # unit 18: 64 nodes, k-major
TAIL_NODE0 = 1216                # unit 19: 34 nodes (1088 cols), k-minor
TAIL_COLS = (NODES_PC - TAIL_NODE0) * K

# walrus codegen rejects Pool-engine TensorTensor ("engine check failed"),
# so the GpSimd L1 offload is unavailable; default GPS empty.
V3B = {int(x) for x in os.environ.get("AGG_V3B", "0,2,4,6,8").split(",") if x != ""}
GPS = {int(x) for x in os.environ.get("AGG_GPS", "").split(",") if x != ""}
WARMUP_MM = int(os.environ.get("AGG_WARMUP_MM", "7"))
BLOCKS = [2048, 4096, 8192, 8192, 8192, 4096, 2048, 2048, 1088]
assert sum(BLOCKS) == ROWS_PC


def _perm():
    """new-column -> original-column permutation (see module docstring)."""
    p = np.empty(ROWS_PC, dtype=np.int64)
    for pr in range(N_PAIRS):
        base = pr * 2 * UNIT
        k = np.arange(K)[:, None]          # 32
        n = np.arange(128)[None, :]        # nodes in pair
        p[base : base + 2 * UNIT] = ((pr * 128 + n) * K + k).reshape(-1)
    base = N_PAIRS * 2 * UNIT
    k = np.arange(K)[:, None]
    n = np.arange(64)[None, :]
    p[base : base + UNIT] = ((SINGLE_NODE0 + n) * K + k).reshape(-1)
    base += UNIT
    n = np.arange(NODES_PC - TAIL_NODE0)[:, None]
    k = np.arange(K)[None, :]
    p[base:] = ((TAIL_NODE0 + n) * K + k).reshape(-1)
    return p


def _build_nc():
    import concourse.bacc as bacc
    import concourse.mybir as mybir
    import concourse.tile as tile

    f32 = mybir.dt.float32
    f16 = mybir.dt.float16
    bf16 = mybir.dt.bfloat16
    AX = mybir.AxisListType
    ALU = mybir.AluOpType
    AF = mybir.ActivationFunctionType

    nc = bacc.Bacc(target_bir_lowering=False)

    nbrT = nc.declare_dram_parameter("nbrT", [DIM, ROWS_PC], bf16, isOutput=False)
    srcT = nc.declare_dram_parameter("srcT", [DIM, NODES_PC], f16, isOutput=False)
    w_pool = nc.declare_dram_parameter("w_pool", [DIM, DIM], bf16, isOutput=False)
    wtop = nc.declare_dram_parameter("wtop", [DIM, OUT], f16, isOutput=False)
    wbot = nc.declare_dram_parameter("wbot", [DIM, OUT], f16, isOutput=False)
    bias_pool = nc.declare_dram_parameter("bias_pool", [DIM], f32, isOutput=False)
    bias = nc.declare_dram_parameter("bias", [OUT], f32, isOutput=False)
    outT = nc.declare_dram_parameter("outT", [OUT, NODES_PC], f32, isOutput=True)

    # GEMM units: 19 x 2048 + tail
    units = []
    c = 0
    while c < ROWS_PC:
        n_c = min(UNIT, ROWS_PC - c)
        units.append((c, n_c))
        c += n_c
    n_units = len(units)

    block_cols = []
    c = 0
    for b in BLOCKS:
        block_cols.append((c, b))
        c += b
    unit_block = []
    for u0, u_n in units:
        for bi, (b0, b_n) in enumerate(block_cols):
            if b0 <= u0 + u_n - 1 < b0 + b_n:
                unit_block.append(bi)
                break

    # phase-2 chunks: (node0, n, after_pair) — after_pair -1 = end
    chunks = [(0, 512, 4), (512, 512, 8), (1024, 226, -1)]

    with tile.TileContext(nc) as tc:
        with (
            tc.tile_pool(name="consts", bufs=1) as consts,
            tc.tile_pool(name="persist", bufs=1) as persist,
            tc.tile_pool(name="xt", bufs=len(BLOCKS)) as xt_pool,
            tc.tile_pool(name="h16", bufs=2) as h16_pool,
            tc.tile_pool(name="tree", bufs=2) as tree_pool,
            tc.tile_pool(name="prelu", bufs=2) as prelu_pool,
            tc.tile_pool(name="outio", bufs=2) as outio,
            tc.tile_pool(name="ps", bufs=2, space="PSUM") as ps_pool,
        ):
            wp_sb = consts.tile([DIM, DIM], bf16)
            nc.sync.dma_start(out=wp_sb, in_=w_pool[:, :])

            xt_tiles = []
            for b0, b_n in block_cols:
                xT = xt_pool.tile([DIM, b_n], bf16)
                nc.sync.dma_start(out=xT, in_=nbrT[:, b0 : b0 + b_n])
                xt_tiles.append(xT)

            srcT_sb = persist.tile([DIM, NODES_PC], f16)
            nc.sync.dma_start(out=srcT_sb, in_=srcT[:, :])
            wtop_sb = consts.tile([DIM, OUT], f16)
            nc.sync.dma_start(out=wtop_sb, in_=wtop[:, :])
            wbot_sb = consts.tile([DIM, OUT], f16)
            nc.sync.dma_start(out=wbot_sb, in_=wbot[:, :])
            bp_sb = consts.tile([DIM, 1], f32)
            nc.sync.dma_start(out=bp_sb, in_=bias_pool.rearrange("(d o) -> d o", o=1))
            bias_sb = consts.tile([OUT, 1], f32)
            nc.sync.dma_start(out=bias_sb, in_=bias.rearrange("(d o) -> d o", o=1))

            pooledT = persist.tile([DIM, NODES_PC], f16)  # raw max scores ^T

            if WARMUP_MM:
                dummy = consts.tile([DIM, MM], bf16)
                nc.vector.memset(dummy, 0.0)
                # all warmups hit ONE tile: back-to-back on the PE FIFO
                # instead of serializing through the pool rotation
                wps = ps_pool.tile([DIM, UNIT], f32, tag="ht")
                for _ in range(WARMUP_MM):
                    nc.tensor.matmul(
                        out=wps[:, :MM], lhsT=wp_sb, rhs=dummy,
                        start=True, stop=True,
                    )

            gps_tm = getattr(nc.gpsimd, "tensor_max", None)

            def gemm_unit(u):
                u0, u_n = units[u]
                bi = unit_block[u]
                xT = xt_tiles[bi]
                b0 = block_cols[bi][0]
                ps = ps_pool.tile([DIM, UNIT], f32, tag="ht")
                for m0 in range(0, u_n, MM):
                    mw = min(MM, u_n - m0)
                    nc.tensor.matmul(
                        out=ps[:, m0 : m0 + mw],
                        lhsT=wp_sb,
                        rhs=xT[:, u0 - b0 + m0 : u0 - b0 + m0 + mw],
                        start=True, stop=True,
                    )
                return ps

            def flat_tree(src_ap, width, node0):
                """Flat halving tensor_max tree from [DIM, width] down to
                width//32 pooled columns (k-major layout)."""
                tr = tree_pool.tile([DIM, 2 * UNIT - 128], f16)
                cur = src_ap
                w = width
                off = 0
                while w > 2 * (width // K):
                    nxt = tr[:, off : off + w // 2]
                    nc.vector.tensor_max(
                        out=nxt, in0=cur[:, : w // 2], in1=cur[:, w // 2 : w]
                    )
                    off += w // 2
                    cur = nxt
                    w //= 2
                nc.vector.tensor_max(
                    out=pooledT[:, node0 : node0 + w // 2],
                    in0=cur[:, : w // 2], in1=cur[:, w // 2 : w],
                )

            def emit_chunk(c0, n_c):
                pr = prelu_pool.tile([DIM, 512], f16)
                nc.vector.tensor_scalar(
                    out=pr[:, :n_c], in0=pooledT[:, c0 : c0 + n_c],
                    scalar1=bp_sb[:, :], scalar2=0.0,
                    op0=ALU.add, op1=ALU.max,
                )
                ops = ps_pool.tile([DIM, UNIT], f32, tag="ht")
                nc.tensor.matmul(
                    out=ops[:, :n_c], lhsT=wtop_sb, rhs=srcT_sb[:, c0 : c0 + n_c],
                    start=True, stop=False,
                )
                nc.tensor.matmul(
                    out=ops[:, :n_c], lhsT=wbot_sb, rhs=pr[:, :n_c],
                    start=False, stop=True,
                )
                o_sb = outio.tile([OUT, 512], f32)
                nc.vector.tensor_scalar(
                    out=o_sb[:, :n_c], in0=ops[:, :n_c],
                    scalar1=bias_sb[:, :], scalar2=0.0,
                    op0=ALU.add, op1=ALU.max,
                )
                nc.sync.dma_start(out=outT[:, c0 : c0 + n_c], in_=o_sb[:, :n_c])

            chunk_i = 0
            for p in range(N_PAIRS):
                psA = gemm_unit(2 * p)
                psB = gemm_unit(2 * p + 1)
                node0 = p * 128
                if p in V3B:
                    h16b = h16_pool.tile([DIM, UNIT], f16, tag="hb")
                    nc.scalar.activation(out=h16b, in_=psB[:, :], func=AF.Copy)
                    tr = tree_pool.tile([DIM, 2 * UNIT - 128], f16)
                    t1 = tr[:, :UNIT]
                    nc.vector.tensor_max(out=t1, in0=psA[:, :], in1=h16b)
                    # remaining flat levels: 1024..128
                    cur, w, off = t1, UNIT, UNIT
                    while w > 256:
                        nxt = tr[:, off : off + w // 2]
                        nc.vector.tensor_max(
                            out=nxt, in0=cur[:, : w // 2], in1=cur[:, w // 2 : w]
                        )
                        off += w // 2
                        cur = nxt
                        w //= 2
                    nc.vector.tensor_max(
                        out=pooledT[:, node0 : node0 + 128],
                        in0=cur[:, :128], in1=cur[:, 128:256],
                    )
                else:
                    h16 = h16_pool.tile([DIM, 2 * UNIT], f16, tag="ha")
                    nc.scalar.activation(out=h16[:, :UNIT], in_=psA[:, :], func=AF.Copy)
                    nc.scalar.activation(out=h16[:, UNIT:], in_=psB[:, :], func=AF.Copy)
                    if p in GPS and gps_tm is not None:
                        tr = tree_pool.tile([DIM, 2 * UNIT - 128], f16)
                        t1 = tr[:, :UNIT]
                        gps_tm(out=t1, in0=h16[:, :UNIT], in1=h16[:, UNIT:])
                        cur, w, off = t1, UNIT, UNIT
                        while w > 256:
                            nxt = tr[:, off : off + w // 2]
                            nc.vector.tensor_max(
                                out=nxt, in0=cur[:, : w // 2], in1=cur[:, w // 2 : w]
                            )
                            off += w // 2
                            cur = nxt
                            w //= 2
                        nc.vector.tensor_max(
                            out=pooledT[:, node0 : node0 + 128],
                            in0=cur[:, :128], in1=cur[:, 128:256],
                        )
                    else:
                        flat_tree(h16, 2 * UNIT, node0)
                while chunk_i < len(chunks) and chunks[chunk_i][2] == p + 1:
                    c0, n_c, _ = chunks[chunk_i]
                    emit_chunk(c0, n_c)
                    chunk_i += 1

            # single unit 18 (k-major, 64 nodes)
            ps = gemm_unit(18)
            h2 = h16_pool.tile([DIM, 2 * UNIT], f16, tag="ha")
            nc.scalar.activation(out=h2[:, :UNIT], in_=ps[:, :], func=AF.Copy)
            flat_tree(h2[:, :UNIT], UNIT, SINGLE_NODE0)

            # tail unit 19 (k-minor, direct reduce)
            ps = gemm_unit(19)
            nn = TAIL_COLS // K
            nc.vector.reduce_max(
                out=pooledT[:, TAIL_NODE0 : TAIL_NODE0 + nn],
                in_=ps[:, :TAIL_COLS].rearrange("p (n k) -> p n k", k=K),
                axis=AX.X,
            )

            for c0, n_c, after in chunks:
                if after == -1:
                    emit_chunk(c0, n_c)

    nc.compile()
    return nc


_NC_CACHE = None
_PERM = None


def _make_in_maps(inputs):
    import ml_dtypes

    global _PERM
    if _PERM is None:
        _PERM = _perm()
    f16 = np.float16
    bf16 = ml_dtypes.bfloat16
    src = np.asarray(inputs["src_features"])
    nbr = np.asarray(inputs["neighbor_features"])
    w_pool = np.asarray(inputs["w_pool"]).astype(bf16)
    w = np.asarray(inputs["w"])
    wtop = w[:DIM].astype(f16)
    wbot = w[DIM:].astype(f16)
    bias_pool = np.ascontiguousarray(inputs["bias_pool"], dtype=np.float32)
    bias = np.ascontiguousarray(inputs["bias"], dtype=np.float32)

    in_maps = []
    for c in range(N_CORES):
        nbrT_c = nbr[c * ROWS_PC : (c + 1) * ROWS_PC].astype(bf16)[_PERM].T
        in_maps.append(
            {
                "nbrT": np.ascontiguousarray(nbrT_c),
                "srcT": src[c * NODES_PC : (c + 1) * NODES_PC].T.astype(f16),
                "w_pool": w_pool,
                "wtop": wtop,
                "wbot": wbot,
                "bias_pool": bias_pool,
                "bias": bias,
            }
        )
    return in_maps


def kernel(**inputs: np.ndarray) -> np.ndarray:
    from concourse.bass_utils import run_bass_kernel_spmd

    global _NC_CACHE
    if _NC_CACHE is None:
        _NC_CACHE = _build_nc()
    nc = _NC_CACHE

    in_maps = _make_in_maps(inputs)
    res = run_bass_kernel_spmd(nc, in_maps, core_ids=list(range(N_CORES)))
    return np.concatenate(
        [np.ascontiguousarray(np.asarray(res.results[c]["outT"]).T) for c in range(N_CORES)],
        axis=0,
    )
